# revision 46
# baseline (speedup 1.0000x reference)
"""Trainium2 Bass kernel for LoRA-dense GQA attention (B=2, S=2048, HID=2048,
H=16, KV=4, D=128, RANK=8).

Sharding: 8 cores = 2 (batch) x 4 (head-group). Each core owns 4 q-heads and
their single shared kv-head, computes Q/K/V projections feature-major
(qT = W.T @ X.T so the head dim lands on partitions), applies RoPE via a
host-side even/odd head-dim permutation (scores are invariant to a shared
permutation of q/k features), runs causal attention in transposed orientation
(scoresT[k,q]) so no on-device transposes are ever needed.

O projection is reformulated for overlap: each core computes a PARTIAL
O-projection over ALL 2048 output features from its local 512 head-features
(per 512-token query chunk, right after that chunk's attention), then a
per-chunk ReduceScatter over the 4 head-group cores sums the partials and
leaves each core its own 512-feature output slice. The collectives pipeline
under the next chunk's attention instead of a single exposed AllGather.
"""

import os
import sys

sys.path.insert(0, "/opt/trn_rl_repo")

import numpy as np
import ml_dtypes

BF16 = np.dtype(ml_dtypes.bfloat16)

B = 2
S = 2048
HID = 2048
H = 16
KVH = 4
D = 128
RANK = 8
ALPHA = 16.0
THETA = 10000.0
MAX_POS = 4096

N_CORES = 8
HG = 4  # head-group (mp) factor
HPC = H // HG  # q heads per core = 4
QF = HPC * D  # per-core q feature slice = 512
KS = HID // 128  # contraction subtiles = 16
TC = 512  # token chunk (matmul free dim)
NT = S // TC  # token chunks = 4
NKT = S // 128  # key tiles = 16
NOT = HID // 128  # O-proj output tiles = 16

_PROG = None  # (nc,) built once per process


def _build_program():
    import concourse.mybir as mybir
    import concourse.tile as tile
    from concourse import bacc
    from concourse.bass import ts

    dt = mybir.dt
    f32 = dt.float32
    bf = dt.bfloat16

    nc = bacc.Bacc("TRN2", target_bir_lowering=False, debug=False,
                   num_devices=N_CORES)

    # ---- I/O ----
    xT = nc.declare_dram_parameter("xT", [KS, 128, S], bf, isOutput=False)
    wq = nc.declare_dram_parameter("wq", [KS, 128, QF], bf, isOutput=False)
    wk = nc.declare_dram_parameter("wk", [KS, 128, D], bf, isOutput=False)
    wv = nc.declare_dram_parameter("wv", [KS, 128, D], bf, isOutput=False)
    # O-proj: local 512 head-features (4 tiles) x ALL 2048 output features
    wo = nc.declare_dram_parameter("wo", [HPC, 128, HID], bf, isOutput=False)
    aqkv = nc.declare_dram_parameter("aqkv", [KS, 128, 96], bf, isOutput=False)
    ao = nc.declare_dram_parameter("ao", [HPC, 128, RANK], bf, isOutput=False)
    wqb = nc.declare_dram_parameter("wqb", [RANK, QF], bf, isOutput=False)
    wkb = nc.declare_dram_parameter("wkb", [RANK, D], bf, isOutput=False)
    wvb = nc.declare_dram_parameter("wvb", [RANK, D], bf, isOutput=False)
    wob = nc.declare_dram_parameter("wob", [RANK, HID], bf, isOutput=False)
    cosd = nc.declare_dram_parameter("cosd", [128, S], f32, isOutput=False)
    sind = nc.declare_dram_parameter("sind", [128, S], f32, isOutput=False)
    masks = nc.declare_dram_parameter("masks", [HPC, 128, TC], bf, isOutput=False)
    # chunk-major so each per-chunk ReduceScatter writes a contiguous block
    yT = nc.declare_dram_parameter("yT", [NT, HPC, 128, TC], bf, isOutput=True)


    # work items: (token offset, width). Last 512-token chunk is split so the
    # final (exposed) ReduceScatter is half-size.
    ITEMS = [(0, TC), (TC, TC), (2 * TC, TC), (3 * TC, TC // 2),
             (3 * TC + TC // 2, TC // 2)]
    # per-item O-proj partials, rank-major so RS hands rank g its quarter
    parts = [nc.dram_tensor(f"part{i}", [HG, HPC, 128, w], bf)
             for i, (_, w) in enumerate(ITEMS)]
    y_rss = [nc.dram_tensor(f"y_rs{i}", [HPC, 128, w], bf)
             for i, (_, w) in enumerate(ITEMS)]

    groups = [list(range(HG)), list(range(HG, N_CORES))]

    with tile.TileContext(nc) as tc:
        with tc.tile_pool(name="persist", bufs=1) as pers:
            mask_sb = pers.tile([128, HPC, TC], bf, tag="masks", name="masks")
            ones_sb = pers.tile([128, 128], bf, tag="ones", name="ones")
            nc.vector.memset(ones_sb[:], 1.0)

            qrot = [pers.tile([128, S], bf, tag=f"qrot{h}", name=f"qrot{h}")
                    for h in range(HPC)]
            krot = pers.tile([128, S], bf, tag="krot", name="krot")
            v_sb = pers.tile([128, NKT, 128], bf, tag="vsb", name="vsb")

            wob_sb = pers.tile([RANK, HID], bf, tag="wob", name="wob")

            # ================= Phase A: projections + RoPE =================
            with tc.tile_pool(name="xw", bufs=1) as xw, \
                 tc.tile_pool(name="ppq", bufs=3, space="PSUM") as ppq, \
                 tc.tile_pool(name="ppv", bufs=2, space="PSUM") as ppv, \
                 tc.tile_pool(name="ppa", bufs=2, space="PSUM") as ppa, \
                 tc.tile_pool(name="rp", bufs=4) as rp:
                # weights first (small), then x in token-chunk passes issued
                # from the idle GpSimd sequencer (sub-tile hazards let each
                # projection chain start as soon as its pass has landed)
                wk_all = xw.tile([128, KS, D], bf, tag="wkall", name="wkall")
                nc.sync.dma_start(wk_all[:], wk.rearrange("s p d -> p s d"))
                wk_sb = [wk_all[:, s, :] for s in range(KS)]
                aq_all = xw.tile([128, KS, 96], bf, tag="aqall", name="aqall")
                nc.sync.dma_start(aq_all[:], aqkv.rearrange("s p d -> p s d"))
                aq_sb = [aq_all[:, s, :] for s in range(KS)]
                wq_all = xw.tile([128, KS, QF], bf, tag="wqall", name="wqall")
                nc.sync.dma_start(wq_all[:], wq.rearrange("s p d -> p s d"))
                wq_sb = [wq_all[:, s, :] for s in range(KS)]
                wv_all = xw.tile([128, KS, D], bf, tag="wvall", name="wvall")
                nc.sync.dma_start(wv_all[:], wv.rearrange("s p d -> p s d"))
                wv_sb = [wv_all[:, s, :] for s in range(KS)]
                wqb_sb = xw.tile([RANK, QF], bf, tag="wqb", name="wqb")
                wkb_sb = xw.tile([RANK, D], bf, tag="wkb", name="wkb")
                wvb_sb = xw.tile([RANK, D], bf, tag="wvb", name="wvb")
                nc.sync.dma_start(wqb_sb[:], wqb[:])
                nc.sync.dma_start(wkb_sb[:], wkb[:])
                nc.sync.dma_start(wvb_sb[:], wvb[:])
                cos_sb = xw.tile([128, S], f32, tag="cos", name="cos")
                sin_sb = xw.tile([128, S], f32, tag="sin", name="sin")
                nc.sync.dma_start(cos_sb[:], cosd[:])
                nc.sync.dma_start(sin_sb[:], sind[:])
                nc.sync.dma_start(mask_sb[:], masks.rearrange("m p t -> p m t"))
                xt = [xw.tile([128, S], bf, tag=f"xt{s}", name=f"xt{s}")
                      for s in range(KS)]
                for t in range(NT):
                    for s in range(KS):
                        nc.gpsimd.dma_start(xt[s][:, ts(t, TC)],
                                            xT[s][:, ts(t, TC)])
                # fused-phase weights last (not needed for ~100us)
                wo_all = pers.tile([128, HPC, HID], bf, tag="woall", name="woall")
                nc.sync.dma_start(wo_all[:], wo.rearrange("h p d -> p h d"))
                ao_all = pers.tile([128, HPC, RANK], bf, tag="aoall", name="aoall")
                nc.sync.dma_start(ao_all[:], ao.rearrange("h p d -> p h d"))
                nc.sync.dma_start(wob_sb[:], wob[:])

                tq = xw.tile([RANK, S], bf, tag="tq", name="tq")
                tk = xw.tile([RANK, S], bf, tag="tk", name="tk")
                tv = xw.tile([RANK, S], bf, tag="tv", name="tv")

                def rope(ps, dest, t):
                    # dest[:, chunk t] = rope(ps) given de-interleaved feature
                    # order (rows 0:63 even dims, 64:127 odd dims).
                    # sin_sb rows 0:63 hold -sin, rows 64:127 hold +sin.
                    a = rp.tile([128, TC], bf, tag="ropeA", name="ropeA")
                    b = rp.tile([128, TC], bf, tag="ropeB", name="ropeB")
                    nc.vector.tensor_mul(out=a[:], in0=ps[:], in1=cos_sb[:, ts(t, TC)])
                    nc.vector.tensor_mul(out=b[0:64, :], in0=ps[64:128, :],
                                         in1=sin_sb[0:64, ts(t, TC)])
                    nc.vector.tensor_mul(out=b[64:128, :], in0=ps[0:64, :],
                                         in1=sin_sb[64:128, ts(t, TC)])
                    nc.vector.tensor_add(out=dest[:, ts(t, TC)], in0=a[:], in1=b[:])

                # token-pass-outer so each pass's chains chase its x DMAs
                for t in range(NT):
                    # LoRA A for q/k/v: [96, tok] (32-aligned packs)
                    ps = ppa.tile([96, TC], f32, tag="pa", name="pa")
                    for s in range(KS):
                        nc.tensor.matmul(ps[:], aq_sb[s][:], xt[s][:, ts(t, TC)],
                                         start=(s == 0), stop=(s == KS - 1))
                    nc.vector.tensor_copy(out=tq[:, ts(t, TC)], in_=ps[0:RANK, :])
                    nc.vector.tensor_copy(out=tk[:, ts(t, TC)],
                                          in_=ps[32:32 + RANK, :])
                    nc.vector.tensor_copy(out=tv[:, ts(t, TC)],
                                          in_=ps[64:64 + RANK, :])

                    # K projection + RoPE
                    ps = ppq.tile([128, TC], f32, tag="pq", name="pq")
                    for s in range(KS):
                        nc.tensor.matmul(ps[:], wk_sb[s][:], xt[s][:, ts(t, TC)],
                                         start=(s == 0), stop=False)
                    nc.tensor.matmul(ps[:], wkb_sb[:], tk[:, ts(t, TC)],
                                     start=False, stop=True)
                    rope(ps, krot, t)

                    # Q projection (feature-major) + RoPE
                    for h in range(HPC):
                        ps = ppq.tile([128, TC], f32, tag="pq", name="pq")
                        for s in range(KS):
                            nc.tensor.matmul(ps[:], wq_sb[s][:, ts(h, 128)],
                                             xt[s][:, ts(t, TC)],
                                             start=(s == 0), stop=False)
                        nc.tensor.matmul(ps[:], wqb_sb[:, ts(h, 128)],
                                         tq[:, ts(t, TC)], start=False, stop=True)
                        rope(ps, qrot[h], t)

                    # V projection, token-major: V[tok, d] = X @ Wv
                    for tt in range(4 * t, 4 * t + 4):
                        ps = ppv.tile([128, 128], f32, tag="pv", name="pv")
                        for s in range(KS):
                            nc.tensor.matmul(ps[:], xt[s][:, ts(tt, 128)],
                                             wv_sb[s][:],
                                             start=(s == 0), stop=False)
                        nc.tensor.matmul(ps[:], tv[:, ts(tt, 128)], wvb_sb[:],
                                         start=False, stop=True)
                        nc.vector.tensor_copy(out=v_sb[:, tt, :], in_=ps[:])

            # ========== Phase B+C fused: attention + partial O + RS =========
            import contextlib
            # softmax-denominator engine: 'pe' (ones-matmul) or 'gpsimd' fold
            sm_mode = os.environ.get("BASS_SM_MODE", "fold")
            sm_dve = sm_mode != "pe"
            stack = contextlib.ExitStack()
            psc = stack.enter_context(
                tc.tile_pool(name="psc", bufs=(3 if sm_dve else 2),
                             space="PSUM"))
            psm = stack.enter_context(
                tc.tile_pool(name="psm", bufs=1, space="PSUM"))
            with stack, \
                 tc.tile_pool(name="tr", bufs=8) as tr, \
                 tc.tile_pool(name="pav", bufs=2, space="PSUM") as pav, \
                 tc.tile_pool(name="ppy", bufs=2, space="PSUM") as ppy, \
                 tc.tile_pool(name="ep", bufs=4) as ep, \
                 tc.tile_pool(name="smp", bufs=2) as smp, \
                 tc.tile_pool(name="oq", bufs=2) as oq, \
                 tc.tile_pool(name="pb", bufs=2) as pb:
                for i, (off, w) in enumerate(ITEMS):
                    nkt = (off + w) // 128  # causal: k tiles 0 .. nkt-1
                    kd = off // 128  # first diagonal k-tile
                    outq = [oq.tile([128, TC], bf, tag=f"outq{h}",
                                    name=f"outq{h}") for h in range(HPC)]
                    for h in range(HPC):
                        av = pav.tile([128, TC], f32, tag="av", name="av")
                        sm = None if sm_dve else psm.tile([128, TC], f32,
                                                          tag="sm", name="sm")
                        pend = {}
                        # streaming binary-counter tree fold of exp sums on
                        # DVE (bf16, non-aliasing adds)
                        pend = {}

                        def fold(node, lvl):
                            while lvl in pend:
                                other = pend.pop(lvl)
                                nt = tr.tile([128, TC], bf, tag="tr", name="tr")
                                nc.vector.tensor_add(out=nt[:, 0:w],
                                                     in0=other[:, 0:w],
                                                     in1=node[:, 0:w])
                                node, lvl = nt, lvl + 1
                            pend[lvl] = node

                        for kt in range(nkt):
                            sc = psc.tile([128, TC], f32, tag="sc", name="sc")
                            nc.tensor.matmul(sc[:, 0:w], krot[:, ts(kt, 128)],
                                             qrot[h][:, off:off + w],
                                             start=True, stop=True)
                            ex = ep.tile([128, TC], bf, tag="ex", name="ex")
                            nc.scalar.activation(ex[:, 0:w], sc[:, 0:w],
                                                 mybir.ActivationFunctionType.Exp)
                            dd = kt - kd
                            if dd >= 0:  # diagonal block: triangular 0/1 mask
                                nc.vector.tensor_mul(out=ex[:, 0:w],
                                                     in0=ex[:, 0:w],
                                                     in1=mask_sb[:, dd, 0:w])
                            nc.tensor.matmul(av[:, 0:w], v_sb[:, kt, :],
                                             ex[:, 0:w],
                                             start=(kt == 0), stop=(kt == nkt - 1))
                            if sm_dve:
                                fold(ex, 0)
                            else:
                                nc.tensor.matmul(sm[:, 0:w], ones_sb[:],
                                                 ex[:, 0:w], start=(kt == 0),
                                                 stop=(kt == nkt - 1))
                        rc = ep.tile([128, TC], f32, tag="rc", name="rc")
                        if sm_dve:
                            # fold leftovers: T[k,q] = sum_kt ex[kt][k,q], then
                            # ONE ones-matmul for the cross-partition key sum
                            rem = [pend[l] for l in sorted(pend)]
                            node = rem[0]
                            for other in rem[1:]:
                                nt = tr.tile([128, TC], bf, tag="tr", name="tr")
                                nc.vector.tensor_add(out=nt[:, 0:w],
                                                     in0=node[:, 0:w],
                                                     in1=other[:, 0:w])
                                node = nt
                            sm = psm.tile([128, TC], f32, tag="sm", name="sm")
                            nc.tensor.matmul(sm[:, 0:w], ones_sb[:],
                                             node[:, 0:w], start=True, stop=True)
                        nc.vector.reciprocal_approx_fast(rc[:, 0:w], sm[:, 0:w])
                        nc.vector.tensor_mul(out=outq[h][:, 0:w],
                                             in0=av[:, 0:w], in1=rc[:, 0:w])

                    # O LoRA-A for this chunk: to = Ao_local.T @ out_local
                    # (borrows the short-lived psm bank)
                    tps = psm.tile([128, TC], f32, tag="sm", name="sm")
                    for hb in range(HPC):
                        nc.tensor.matmul(tps[0:RANK, 0:w], ao_all[:, hb, :],
                                         outq[hb][:, 0:w],
                                         start=(hb == 0), stop=(hb == HPC - 1))
                    to_sb = ep.tile([RANK, TC], bf, tag="to", name="to")
                    nc.vector.tensor_copy(out=to_sb[:, 0:w], in_=tps[0:RANK, 0:w])

                    # partial O-projection: all 2048 out feats, local contraction
                    pso = pb.tile([128, NOT, TC], bf, tag="pso", name="pso")
                    for ot in range(NOT):
                        g, j = divmod(ot, HPC)
                        py = ppy.tile([128, TC], f32, tag="py", name="py")
                        for hb in range(HPC):
                            nc.tensor.matmul(py[:, 0:w], wo_all[:, hb, ts(ot, 128)],
                                             outq[hb][:, 0:w],
                                             start=(hb == 0), stop=False)
                        nc.tensor.matmul(py[:, 0:w], wob_sb[:, ts(ot, 128)],
                                         to_sb[:, 0:w], start=False, stop=True)
                        nc.vector.tensor_copy(out=pso[:, ot, 0:w], in_=py[:, 0:w])
                        nc.gpsimd.dma_start(parts[i][g, j], pso[:, ot, 0:w])
                    # ReduceScatter this item's partials; rank g gets quarter g
                    nc.gpsimd.collective_compute(
                        "ReduceScatter", mybir.AluOpType.add,
                        replica_groups=groups,
                        ins=[parts[i][:]], outs=[y_rss[i][:]],
                    )
                    qi, so = off // TC, off % TC
                    nc.gpsimd.dma_start(yT[qi, :, :, so:so + w], y_rss[i][:])

    nc.finalize()
    return nc


def _host_inputs(hidden_states, position_ids, wq_kernel, wq_a, wq_b, wk_kernel,
                 wk_a, wk_b, wv_kernel, wv_a, wv_b, wo_kernel, wo_a, wo_b):
    """Build the 8 per-core input maps."""
    perm = np.concatenate([np.arange(0, D, 2), np.arange(1, D, 2)])
    scale = np.float32(1.0 / np.sqrt(D))
    lora = np.float32(ALPHA / RANK)

    def bfc(x):
        return np.ascontiguousarray(np.asarray(x, np.float32)).astype(BF16)

    # rope tables per batch
    inv_freq = (1.0 / THETA ** (np.arange(0, D, 2, dtype=np.float32)[: D // 2]
                                / np.float32(D)))
    angles = np.arange(MAX_POS, dtype=np.float32)[:, None] * inv_freq[None, :]
    cos_t = np.cos(angles).astype(np.float32)
    sin_t = np.sin(angles).astype(np.float32)

    # causal 0/1 masks for the 4 diagonal block offsets
    masks = np.zeros((HPC, 128, TC), np.float32)
    qq = np.arange(TC)[None, :]
    kk = np.arange(128)[:, None]
    for dd in range(HPC):
        masks[dd] = (dd * 128 + kk <= qq).astype(np.float32)
    masks = masks.astype(BF16)

    aqkv = np.zeros((HID, 96), np.float32)
    aqkv[:, 0:RANK] = wq_a
    aqkv[:, 32:32 + RANK] = wk_a
    aqkv[:, 64:64 + RANK] = wv_a
    aqkv = bfc(aqkv).reshape(KS, 128, 96)

    in_maps = []
    for c in range(N_CORES):
        b, hg = divmod(c, HG)
        xTb = bfc(hidden_states[b]).T  # [HID, S] bf16
        xTb = np.ascontiguousarray(xTb).reshape(KS, 128, S)

        wq_c = np.asarray(wq_kernel[:, hg * QF:(hg + 1) * QF], np.float32)
        wq_c = wq_c.reshape(HID, HPC, D)[:, :, perm].reshape(HID, QF) * scale
        wq_c = bfc(wq_c).reshape(KS, 128, QF)
        wk_c = np.asarray(wk_kernel[:, hg * D:(hg + 1) * D], np.float32)[:, perm]
        wk_c = bfc(wk_c).reshape(KS, 128, D)
        wv_c = bfc(wv_kernel[:, hg * D:(hg + 1) * D]).reshape(KS, 128, D)
        # O-proj: LOCAL head-feature rows x all output features
        wo_c = bfc(wo_kernel[hg * QF:(hg + 1) * QF, :]).reshape(HPC, 128, HID)
        ao_c = bfc(wo_a[hg * QF:(hg + 1) * QF, :]).reshape(HPC, 128, RANK)

        wqb_c = np.asarray(wq_b[:, hg * QF:(hg + 1) * QF], np.float32)
        wqb_c = wqb_c.reshape(RANK, HPC, D)[:, :, perm].reshape(RANK, QF)
        wqb_c = bfc(wqb_c * (lora * scale))
        wkb_c = np.asarray(wk_b[:, hg * D:(hg + 1) * D], np.float32)[:, perm]
        wkb_c = bfc(wkb_c * lora)
        wvb_c = bfc(np.asarray(wv_b[:, hg * D:(hg + 1) * D], np.float32) * lora)
        wob_c = bfc(np.asarray(wo_b, np.float32) * lora)  # full [RANK, HID]

        pos = np.asarray(position_ids[b], np.int64)
        cb = cos_t[pos].T  # [64, S]
        sb = sin_t[pos].T
        cosd = np.ascontiguousarray(np.concatenate([cb, cb], 0), dtype=np.float32)
        sind = np.ascontiguousarray(np.concatenate([-sb, sb], 0), dtype=np.float32)

        in_maps.append({
            "xT": xTb, "wq": wq_c, "wk": wk_c, "wv": wv_c, "wo": wo_c,
            "aqkv": aqkv, "ao": ao_c, "wqb": wqb_c, "wkb": wkb_c,
            "wvb": wvb_c, "wob": wob_c, "cosd": cosd, "sind": sind,
            "masks": masks,
        })
    return in_maps


_last_exec_time_ns = None
_last_results = None


def kernel(hidden_states, attention_mask, position_ids, wq_kernel, wq_a, wq_b,
           wk_kernel, wk_a, wk_b, wv_kernel, wv_a, wv_b, wo_kernel, wo_a, wo_b):
    global _PROG, _last_exec_time_ns
    hidden_states = np.asarray(hidden_states)
    attention_mask = np.asarray(attention_mask)
    if (hidden_states.shape != (B, S, HID)) or not attention_mask.all():
        return _kernel_numpy(hidden_states, attention_mask,
                             np.asarray(position_ids), wq_kernel, wq_a, wq_b,
                             wk_kernel, wk_a, wk_b, wv_kernel, wv_a, wv_b,
                             wo_kernel, wo_a, wo_b)

    from concourse.bass_utils import run_bass_kernel_spmd

    if _PROG is None:
        _PROG = _build_program()
    nc = _PROG

    in_maps = _host_inputs(hidden_states, position_ids, wq_kernel, wq_a, wq_b,
                           wk_kernel, wk_a, wk_b, wv_kernel, wv_a, wv_b,
                           wo_kernel, wo_a, wo_b)
    trace = bool(int(os.environ.get("BASS_KERNEL_TRACE", "0")))
    res = run_bass_kernel_spmd(nc, in_maps, list(range(N_CORES)), trace=trace)
    _last_exec_time_ns = res.exec_time_ns
    global _last_results
    _last_results = res

    out = np.empty((B, S, HID), np.float32)
    for c in range(N_CORES):
        b, hg = divmod(c, HG)
        yT = res.results[c]["yT"].reshape(NT, QF, TC)  # [chunk, feat, tok]
        y = np.concatenate(list(yT), axis=1)  # [feat, S] bf16
        out[b, :, hg * QF:(hg + 1) * QF] = y.T.astype(np.float32)
    return out


# ---------------- numpy fallback (general shapes / masks) ----------------

def _bf16_round(x):
    return np.asarray(x, np.float32).astype(BF16).astype(np.float32)


def _mm_bf16(x, w):
    y = (np.asarray(x, np.float32).astype(BF16).astype(np.float32)
         @ np.asarray(w, np.float32).astype(BF16).astype(np.float32))
    return _bf16_round(y)


def _lora_dense_np(x, kernel_, a, b):
    y = _mm_bf16(x, kernel_)
    t = _mm_bf16(_mm_bf16(x, a), b)
    return _bf16_round(y + _bf16_round((ALPHA / RANK) * t)).astype(np.float32)


def _kernel_numpy(hidden_states, attention_mask, position_ids, wq_kernel, wq_a,
                  wq_b, wk_kernel, wk_a, wk_b, wv_kernel, wv_a, wv_b, wo_kernel,
                  wo_a, wo_b):
    b, s, _ = hidden_states.shape
    x2 = np.asarray(hidden_states, np.float32).reshape(b * s, HID)
    xq = _lora_dense_np(x2, wq_kernel, wq_a, wq_b).reshape(b, s, H, D)
    xk = _lora_dense_np(x2, wk_kernel, wk_a, wk_b).reshape(b, s, KVH, D)
    xv = _lora_dense_np(x2, wv_kernel, wv_a, wv_b).reshape(b, s, KVH, D)
    g = H // KVH
    xk = np.repeat(xk, g, axis=2)
    xv = np.repeat(xv, g, axis=2)

    inv_freq = 1.0 / THETA ** (np.arange(0, D, 2, dtype=np.float32)[: D // 2]
                               / np.float32(D))
    angles = np.arange(MAX_POS, dtype=np.float32)[:, None] * inv_freq[None, :]
    cos = np.cos(angles).astype(np.float32)[position_ids][:, :, None, :]
    sin = np.sin(angles).astype(np.float32)[position_ids][:, :, None, :]

    def rot(x):
        x = _bf16_round(x)
        xe = x[..., 0::2]
        xo = x[..., 1::2]
        re = xe * cos - xo * sin
        im = xe * sin + xo * cos
        return np.stack((re, im), axis=-1).reshape(x.shape)

    xq, xk = rot(xq), rot(xk)
    causal = np.tril(np.ones((s, s), dtype=bool))
    mask = np.asarray(attention_mask)[:, None, None, :] & causal[None, None]
    bias = np.where(mask, np.float32(0.0), np.float32(np.finfo(np.float32).min))
    scale = np.float32(1.0 / np.sqrt(D))
    qs = _bf16_round(xq * scale)
    ks = _bf16_round(xk)
    out = np.empty((b, s, H, D), np.float32)
    for bi in range(b):
        for h in range(H):
            sc = _bf16_round(qs[bi, :, h, :] @ ks[bi, :, h, :].T) + bias[bi, 0]
            sc = sc - sc.max(axis=-1, keepdims=True)
            e = np.exp(sc)
            attn = e / e.sum(axis=-1, keepdims=True)
            out[bi, :, h, :] = attn @ xv[bi, :, h, :]
    out = out.reshape(b * s, H * D)
    return _lora_dense_np(out, wo_kernel, wo_a, wo_b).reshape(b, s, HID)


# revision 49
# speedup vs baseline: 1.0106x; 1.0106x over previous
"""Trainium2 Bass kernel for LoRA-dense GQA attention (B=2, S=2048, HID=2048,
H=16, KV=4, D=128, RANK=8).

Sharding: 8 cores = 2 (batch) x 4 (head-group). Each core owns 4 q-heads and
their single shared kv-head, computes Q/K/V projections feature-major
(qT = W.T @ X.T so the head dim lands on partitions), applies RoPE via a
host-side even/odd head-dim permutation (scores are invariant to a shared
permutation of q/k features), runs causal attention in transposed orientation
(scoresT[k,q]) so no on-device transposes are ever needed.

O projection is reformulated for overlap: each core computes a PARTIAL
O-projection over ALL 2048 output features from its local 512 head-features
(per 512-token query chunk, right after that chunk's attention), then a
per-chunk ReduceScatter over the 4 head-group cores sums the partials and
leaves each core its own 512-feature output slice. The collectives pipeline
under the next chunk's attention instead of a single exposed AllGather.
"""

import os
import sys

sys.path.insert(0, "/opt/trn_rl_repo")

import numpy as np
import ml_dtypes

BF16 = np.dtype(ml_dtypes.bfloat16)

B = 2
S = 2048
HID = 2048
H = 16
KVH = 4
D = 128
RANK = 8
ALPHA = 16.0
THETA = 10000.0
MAX_POS = 4096

N_CORES = 8
HG = 4  # head-group (mp) factor
HPC = H // HG  # q heads per core = 4
QF = HPC * D  # per-core q feature slice = 512
KS = HID // 128  # contraction subtiles = 16
TC = 512  # token chunk (matmul free dim)
NT = S // TC  # token chunks = 4
NKT = S // 128  # key tiles = 16
NOT = HID // 128  # O-proj output tiles = 16

_PROG = None  # (nc,) built once per process


def _build_program():
    import concourse.mybir as mybir
    import concourse.tile as tile
    from concourse import bacc
    from concourse.bass import ts

    dt = mybir.dt
    f32 = dt.float32
    bf = dt.bfloat16

    nc = bacc.Bacc("TRN2", target_bir_lowering=False, debug=False,
                   num_devices=N_CORES)

    # ---- I/O ----
    xT = nc.declare_dram_parameter("xT", [KS, 128, S], bf, isOutput=False)
    wq = nc.declare_dram_parameter("wq", [KS, 128, QF], bf, isOutput=False)
    wk = nc.declare_dram_parameter("wk", [KS, 128, D], bf, isOutput=False)
    wv = nc.declare_dram_parameter("wv", [KS, 128, D], bf, isOutput=False)
    # O-proj: local 512 head-features (4 tiles) x ALL 2048 output features
    wo = nc.declare_dram_parameter("wo", [HPC, 128, HID], bf, isOutput=False)
    aqkv = nc.declare_dram_parameter("aqkv", [KS, 128, 96], bf, isOutput=False)
    ao = nc.declare_dram_parameter("ao", [HPC, 128, RANK], bf, isOutput=False)
    wqb = nc.declare_dram_parameter("wqb", [RANK, QF], bf, isOutput=False)
    wkb = nc.declare_dram_parameter("wkb", [RANK, D], bf, isOutput=False)
    wvb = nc.declare_dram_parameter("wvb", [RANK, D], bf, isOutput=False)
    wob = nc.declare_dram_parameter("wob", [RANK, HID], bf, isOutput=False)
    cosd = nc.declare_dram_parameter("cosd", [128, S], f32, isOutput=False)
    sind = nc.declare_dram_parameter("sind", [128, S], f32, isOutput=False)
    masks = nc.declare_dram_parameter("masks", [HPC, 128, TC], bf, isOutput=False)
    # chunk-major so each per-chunk ReduceScatter writes a contiguous block
    yT = nc.declare_dram_parameter("yT", [NT, HPC, 128, TC], bf, isOutput=True)


    # tiny dummy collective to warm up the CC rings during the DMA preroll
    cc_warm_in = nc.dram_tensor("cc_warm_in", [HG, 128, 4], bf)
    cc_warm_out = nc.dram_tensor("cc_warm_out", [128, 4], bf)
    # work items: (token offset, width). Last 512-token chunk is split so the
    # final (exposed) ReduceScatter is half-size.
    ITEMS = [(0, TC), (TC, TC), (2 * TC, TC), (3 * TC, TC // 2),
             (3 * TC + TC // 2, TC // 2)]
    # per-item O-proj partials, rank-major so RS hands rank g its quarter
    parts = [nc.dram_tensor(f"part{i}", [HG, HPC, 128, w], bf)
             for i, (_, w) in enumerate(ITEMS)]
    y_rss = [nc.dram_tensor(f"y_rs{i}", [HPC, 128, w], bf)
             for i, (_, w) in enumerate(ITEMS)]

    groups = [list(range(HG)), list(range(HG, N_CORES))]

    with tile.TileContext(nc) as tc:
        with tc.tile_pool(name="persist", bufs=1) as pers:
            mask_sb = pers.tile([128, HPC, TC], bf, tag="masks", name="masks")
            ones_sb = pers.tile([128, 128], bf, tag="ones", name="ones")
            nc.vector.memset(ones_sb[:], 1.0)

            qrot = [pers.tile([128, S], bf, tag=f"qrot{h}", name=f"qrot{h}")
                    for h in range(HPC)]
            krot = pers.tile([128, S], bf, tag="krot", name="krot")
            v_sb = pers.tile([128, NKT, 128], bf, tag="vsb", name="vsb")

            wob_sb = pers.tile([RANK, HID], bf, tag="wob", name="wob")

            # ================= Phase A: projections + RoPE =================
            with tc.tile_pool(name="xw", bufs=1) as xw, \
                 tc.tile_pool(name="ppq", bufs=3, space="PSUM") as ppq, \
                 tc.tile_pool(name="ppv", bufs=2, space="PSUM") as ppv, \
                 tc.tile_pool(name="ppa", bufs=2, space="PSUM") as ppa, \
                 tc.tile_pool(name="rp", bufs=4) as rp:
                # weights first (small), then x in token-chunk passes issued
                # from the idle GpSimd sequencer (sub-tile hazards let each
                # projection chain start as soon as its pass has landed)
                wk_all = xw.tile([128, KS, D], bf, tag="wkall", name="wkall")
                nc.sync.dma_start(wk_all[:], wk.rearrange("s p d -> p s d"))
                wk_sb = [wk_all[:, s, :] for s in range(KS)]
                aq_all = xw.tile([128, KS, 96], bf, tag="aqall", name="aqall")
                nc.sync.dma_start(aq_all[:], aqkv.rearrange("s p d -> p s d"))
                aq_sb = [aq_all[:, s, :] for s in range(KS)]
                wq_all = xw.tile([128, KS, QF], bf, tag="wqall", name="wqall")
                nc.sync.dma_start(wq_all[:], wq.rearrange("s p d -> p s d"))
                wq_sb = [wq_all[:, s, :] for s in range(KS)]
                wv_all = xw.tile([128, KS, D], bf, tag="wvall", name="wvall")
                nc.sync.dma_start(wv_all[:], wv.rearrange("s p d -> p s d"))
                wv_sb = [wv_all[:, s, :] for s in range(KS)]
                wqb_sb = xw.tile([RANK, QF], bf, tag="wqb", name="wqb")
                wkb_sb = xw.tile([RANK, D], bf, tag="wkb", name="wkb")
                wvb_sb = xw.tile([RANK, D], bf, tag="wvb", name="wvb")
                nc.sync.dma_start(wqb_sb[:], wqb[:])
                nc.sync.dma_start(wkb_sb[:], wkb[:])
                nc.sync.dma_start(wvb_sb[:], wvb[:])
                cos_sb = xw.tile([128, S], f32, tag="cos", name="cos")
                sin_sb = xw.tile([128, S], f32, tag="sin", name="sin")
                nc.sync.dma_start(cos_sb[:], cosd[:])
                nc.sync.dma_start(sin_sb[:], sind[:])
                nc.sync.dma_start(mask_sb[:], masks.rearrange("m p t -> p m t"))
                # warm up the collective rings while DMA preroll runs
                # (first real ReduceScatter otherwise pays ~14us startup)
                nc.gpsimd.collective_compute(
                    "ReduceScatter", mybir.AluOpType.add,
                    replica_groups=groups,
                    ins=[cc_warm_in[:]], outs=[cc_warm_out[:]],
                )
                xt = [xw.tile([128, S], bf, tag=f"xt{s}", name=f"xt{s}")
                      for s in range(KS)]
                for t in range(NT):
                    for s in range(KS):
                        nc.gpsimd.dma_start(xt[s][:, ts(t, TC)],
                                            xT[s][:, ts(t, TC)])
                # fused-phase weights last (not needed for ~100us)
                wo_all = pers.tile([128, HPC, HID], bf, tag="woall", name="woall")
                nc.sync.dma_start(wo_all[:], wo.rearrange("h p d -> p h d"))
                ao_all = pers.tile([128, HPC, RANK], bf, tag="aoall", name="aoall")
                nc.sync.dma_start(ao_all[:], ao.rearrange("h p d -> p h d"))
                nc.sync.dma_start(wob_sb[:], wob[:])

                tq = xw.tile([RANK, S], bf, tag="tq", name="tq")
                tk = xw.tile([RANK, S], bf, tag="tk", name="tk")
                tv = xw.tile([RANK, S], bf, tag="tv", name="tv")

                def rope(ps, dest, t):
                    # dest[:, chunk t] = rope(ps) given de-interleaved feature
                    # order (rows 0:63 even dims, 64:127 odd dims).
                    # sin_sb rows 0:63 hold -sin, rows 64:127 hold +sin.
                    a = rp.tile([128, TC], bf, tag="ropeA", name="ropeA")
                    b = rp.tile([128, TC], bf, tag="ropeB", name="ropeB")
                    nc.vector.tensor_mul(out=a[:], in0=ps[:], in1=cos_sb[:, ts(t, TC)])
                    nc.vector.tensor_mul(out=b[0:64, :], in0=ps[64:128, :],
                                         in1=sin_sb[0:64, ts(t, TC)])
                    nc.vector.tensor_mul(out=b[64:128, :], in0=ps[0:64, :],
                                         in1=sin_sb[64:128, ts(t, TC)])
                    nc.vector.tensor_add(out=dest[:, ts(t, TC)], in0=a[:], in1=b[:])

                # token-pass-outer so each pass's chains chase its x DMAs
                for t in range(NT):
                    # LoRA A for q/k/v: [96, tok] (32-aligned packs)
                    ps = ppa.tile([96, TC], f32, tag="pa", name="pa")
                    for s in range(KS):
                        nc.tensor.matmul(ps[:], aq_sb[s][:], xt[s][:, ts(t, TC)],
                                         start=(s == 0), stop=(s == KS - 1))
                    nc.vector.tensor_copy(out=tq[:, ts(t, TC)], in_=ps[0:RANK, :])
                    nc.vector.tensor_copy(out=tk[:, ts(t, TC)],
                                          in_=ps[32:32 + RANK, :])
                    nc.vector.tensor_copy(out=tv[:, ts(t, TC)],
                                          in_=ps[64:64 + RANK, :])

                    # K projection + RoPE
                    ps = ppq.tile([128, TC], f32, tag="pq", name="pq")
                    for s in range(KS):
                        nc.tensor.matmul(ps[:], wk_sb[s][:], xt[s][:, ts(t, TC)],
                                         start=(s == 0), stop=False)
                    nc.tensor.matmul(ps[:], wkb_sb[:], tk[:, ts(t, TC)],
                                     start=False, stop=True)
                    rope(ps, krot, t)

                    # Q projection (feature-major) + RoPE
                    for h in range(HPC):
                        ps = ppq.tile([128, TC], f32, tag="pq", name="pq")
                        for s in range(KS):
                            nc.tensor.matmul(ps[:], wq_sb[s][:, ts(h, 128)],
                                             xt[s][:, ts(t, TC)],
                                             start=(s == 0), stop=False)
                        nc.tensor.matmul(ps[:], wqb_sb[:, ts(h, 128)],
                                         tq[:, ts(t, TC)], start=False, stop=True)
                        rope(ps, qrot[h], t)

                    # V projection, token-major: V[tok, d] = X @ Wv
                    for tt in range(4 * t, 4 * t + 4):
                        ps = ppv.tile([128, 128], f32, tag="pv", name="pv")
                        for s in range(KS):
                            nc.tensor.matmul(ps[:], xt[s][:, ts(tt, 128)],
                                             wv_sb[s][:],
                                             start=(s == 0), stop=False)
                        nc.tensor.matmul(ps[:], tv[:, ts(tt, 128)], wvb_sb[:],
                                         start=False, stop=True)
                        nc.vector.tensor_copy(out=v_sb[:, tt, :], in_=ps[:])

            # ========== Phase B+C fused: attention + partial O + RS =========
            import contextlib
            # softmax-denominator engine: 'pe' (ones-matmul) or 'gpsimd' fold
            sm_mode = os.environ.get("BASS_SM_MODE", "fold")
            sm_dve = sm_mode != "pe"
            stack = contextlib.ExitStack()
            psc = stack.enter_context(
                tc.tile_pool(name="psc", bufs=(3 if sm_dve else 2),
                             space="PSUM"))
            psm = stack.enter_context(
                tc.tile_pool(name="psm", bufs=1, space="PSUM"))
            with stack, \
                 tc.tile_pool(name="tr", bufs=8) as tr, \
                 tc.tile_pool(name="pav", bufs=2, space="PSUM") as pav, \
                 tc.tile_pool(name="ppy", bufs=2, space="PSUM") as ppy, \
                 tc.tile_pool(name="ep", bufs=4) as ep, \
                 tc.tile_pool(name="smp", bufs=2) as smp, \
                 tc.tile_pool(name="oq", bufs=2) as oq, \
                 tc.tile_pool(name="pb", bufs=2) as pb:
                for i, (off, w) in enumerate(ITEMS):
                    nkt = (off + w) // 128  # causal: k tiles 0 .. nkt-1
                    kd = off // 128  # first diagonal k-tile
                    outq = [oq.tile([128, TC], bf, tag=f"outq{h}",
                                    name=f"outq{h}") for h in range(HPC)]
                    for h in range(HPC):
                        av = pav.tile([128, TC], f32, tag="av", name="av")
                        sm = None if sm_dve else psm.tile([128, TC], f32,
                                                          tag="sm", name="sm")
                        pend = {}
                        # streaming binary-counter tree fold of exp sums on
                        # DVE (bf16, non-aliasing adds)
                        pend = {}

                        def fold(node, lvl):
                            while lvl in pend:
                                other = pend.pop(lvl)
                                nt = tr.tile([128, TC], bf, tag="tr", name="tr")
                                nc.vector.tensor_add(out=nt[:, 0:w],
                                                     in0=other[:, 0:w],
                                                     in1=node[:, 0:w])
                                node, lvl = nt, lvl + 1
                            pend[lvl] = node

                        for kt in range(nkt):
                            sc = psc.tile([128, TC], f32, tag="sc", name="sc")
                            nc.tensor.matmul(sc[:, 0:w], krot[:, ts(kt, 128)],
                                             qrot[h][:, off:off + w],
                                             start=True, stop=True)
                            ex = ep.tile([128, TC], bf, tag="ex", name="ex")
                            nc.scalar.activation(ex[:, 0:w], sc[:, 0:w],
                                                 mybir.ActivationFunctionType.Exp)
                            dd = kt - kd
                            if dd >= 0:  # diagonal block: triangular 0/1 mask
                                nc.vector.tensor_mul(out=ex[:, 0:w],
                                                     in0=ex[:, 0:w],
                                                     in1=mask_sb[:, dd, 0:w])
                            nc.tensor.matmul(av[:, 0:w], v_sb[:, kt, :],
                                             ex[:, 0:w],
                                             start=(kt == 0), stop=(kt == nkt - 1))
                            if sm_dve:
                                fold(ex, 0)
                            else:
                                nc.tensor.matmul(sm[:, 0:w], ones_sb[:],
                                                 ex[:, 0:w], start=(kt == 0),
                                                 stop=(kt == nkt - 1))
                        rc = ep.tile([128, TC], f32, tag="rc", name="rc")
                        if sm_dve:
                            # fold leftovers: T[k,q] = sum_kt ex[kt][k,q], then
                            # ONE ones-matmul for the cross-partition key sum
                            rem = [pend[l] for l in sorted(pend)]
                            node = rem[0]
                            for other in rem[1:]:
                                nt = tr.tile([128, TC], bf, tag="tr", name="tr")
                                nc.vector.tensor_add(out=nt[:, 0:w],
                                                     in0=node[:, 0:w],
                                                     in1=other[:, 0:w])
                                node = nt
                            sm = psm.tile([128, TC], f32, tag="sm", name="sm")
                            nc.tensor.matmul(sm[:, 0:w], ones_sb[:],
                                             node[:, 0:w], start=True, stop=True)
                        nc.vector.reciprocal_approx_fast(rc[:, 0:w], sm[:, 0:w])
                        nc.vector.tensor_mul(out=outq[h][:, 0:w],
                                             in0=av[:, 0:w], in1=rc[:, 0:w])

                    # O LoRA-A for this chunk: to = Ao_local.T @ out_local
                    # (borrows the short-lived psm bank)
                    tps = psm.tile([128, TC], f32, tag="sm", name="sm")
                    for hb in range(HPC):
                        nc.tensor.matmul(tps[0:RANK, 0:w], ao_all[:, hb, :],
                                         outq[hb][:, 0:w],
                                         start=(hb == 0), stop=(hb == HPC - 1))
                    to_sb = ep.tile([RANK, TC], bf, tag="to", name="to")
                    nc.vector.tensor_copy(out=to_sb[:, 0:w], in_=tps[0:RANK, 0:w])

                    # partial O-projection: all 2048 out feats, local contraction
                    pso = pb.tile([128, NOT, TC], bf, tag="pso", name="pso")
                    for ot in range(NOT):
                        g, j = divmod(ot, HPC)
                        py = ppy.tile([128, TC], f32, tag="py", name="py")
                        for hb in range(HPC):
                            nc.tensor.matmul(py[:, 0:w], wo_all[:, hb, ts(ot, 128)],
                                             outq[hb][:, 0:w],
                                             start=(hb == 0), stop=False)
                        nc.tensor.matmul(py[:, 0:w], wob_sb[:, ts(ot, 128)],
                                         to_sb[:, 0:w], start=False, stop=True)
                        # scalar-engine copy: DVE is the busier engine here
                        nc.scalar.copy(pso[:, ot, 0:w], py[:, 0:w])
                        nc.gpsimd.dma_start(parts[i][g, j], pso[:, ot, 0:w])
                    # ReduceScatter this item's partials; rank g gets quarter g
                    nc.gpsimd.collective_compute(
                        "ReduceScatter", mybir.AluOpType.add,
                        replica_groups=groups,
                        ins=[parts[i][:]], outs=[y_rss[i][:]],
                    )
                    # yT copy waits on RS completion -> keep it OFF the gpsimd
                    # queue so later part-DMAs/RS issues aren't serialized
                    qi, so = off // TC, off % TC
                    nc.sync.dma_start(yT[qi, :, :, so:so + w], y_rss[i][:])

    nc.finalize()
    return nc


def _host_inputs(hidden_states, position_ids, wq_kernel, wq_a, wq_b, wk_kernel,
                 wk_a, wk_b, wv_kernel, wv_a, wv_b, wo_kernel, wo_a, wo_b):
    """Build the 8 per-core input maps."""
    perm = np.concatenate([np.arange(0, D, 2), np.arange(1, D, 2)])
    scale = np.float32(1.0 / np.sqrt(D))
    lora = np.float32(ALPHA / RANK)

    def bfc(x):
        return np.ascontiguousarray(np.asarray(x, np.float32)).astype(BF16)

    # rope tables per batch
    inv_freq = (1.0 / THETA ** (np.arange(0, D, 2, dtype=np.float32)[: D // 2]
                                / np.float32(D)))
    angles = np.arange(MAX_POS, dtype=np.float32)[:, None] * inv_freq[None, :]
    cos_t = np.cos(angles).astype(np.float32)
    sin_t = np.sin(angles).astype(np.float32)

    # causal 0/1 masks for the 4 diagonal block offsets
    masks = np.zeros((HPC, 128, TC), np.float32)
    qq = np.arange(TC)[None, :]
    kk = np.arange(128)[:, None]
    for dd in range(HPC):
        masks[dd] = (dd * 128 + kk <= qq).astype(np.float32)
    masks = masks.astype(BF16)

    aqkv = np.zeros((HID, 96), np.float32)
    aqkv[:, 0:RANK] = wq_a
    aqkv[:, 32:32 + RANK] = wk_a
    aqkv[:, 64:64 + RANK] = wv_a
    aqkv = bfc(aqkv).reshape(KS, 128, 96)

    in_maps = []
    for c in range(N_CORES):
        b, hg = divmod(c, HG)
        xTb = bfc(hidden_states[b]).T  # [HID, S] bf16
        xTb = np.ascontiguousarray(xTb).reshape(KS, 128, S)

        wq_c = np.asarray(wq_kernel[:, hg * QF:(hg + 1) * QF], np.float32)
        wq_c = wq_c.reshape(HID, HPC, D)[:, :, perm].reshape(HID, QF) * scale
        wq_c = bfc(wq_c).reshape(KS, 128, QF)
        wk_c = np.asarray(wk_kernel[:, hg * D:(hg + 1) * D], np.float32)[:, perm]
        wk_c = bfc(wk_c).reshape(KS, 128, D)
        wv_c = bfc(wv_kernel[:, hg * D:(hg + 1) * D]).reshape(KS, 128, D)
        # O-proj: LOCAL head-feature rows x all output features
        wo_c = bfc(wo_kernel[hg * QF:(hg + 1) * QF, :]).reshape(HPC, 128, HID)
        ao_c = bfc(wo_a[hg * QF:(hg + 1) * QF, :]).reshape(HPC, 128, RANK)

        wqb_c = np.asarray(wq_b[:, hg * QF:(hg + 1) * QF], np.float32)
        wqb_c = wqb_c.reshape(RANK, HPC, D)[:, :, perm].reshape(RANK, QF)
        wqb_c = bfc(wqb_c * (lora * scale))
        wkb_c = np.asarray(wk_b[:, hg * D:(hg + 1) * D], np.float32)[:, perm]
        wkb_c = bfc(wkb_c * lora)
        wvb_c = bfc(np.asarray(wv_b[:, hg * D:(hg + 1) * D], np.float32) * lora)
        wob_c = bfc(np.asarray(wo_b, np.float32) * lora)  # full [RANK, HID]

        pos = np.asarray(position_ids[b], np.int64)
        cb = cos_t[pos].T  # [64, S]
        sb = sin_t[pos].T
        cosd = np.ascontiguousarray(np.concatenate([cb, cb], 0), dtype=np.float32)
        sind = np.ascontiguousarray(np.concatenate([-sb, sb], 0), dtype=np.float32)

        in_maps.append({
            "xT": xTb, "wq": wq_c, "wk": wk_c, "wv": wv_c, "wo": wo_c,
            "aqkv": aqkv, "ao": ao_c, "wqb": wqb_c, "wkb": wkb_c,
            "wvb": wvb_c, "wob": wob_c, "cosd": cosd, "sind": sind,
            "masks": masks,
        })
    return in_maps


_last_exec_time_ns = None
_last_results = None


def kernel(hidden_states, attention_mask, position_ids, wq_kernel, wq_a, wq_b,
           wk_kernel, wk_a, wk_b, wv_kernel, wv_a, wv_b, wo_kernel, wo_a, wo_b):
    global _PROG, _last_exec_time_ns
    hidden_states = np.asarray(hidden_states)
    attention_mask = np.asarray(attention_mask)
    if (hidden_states.shape != (B, S, HID)) or not attention_mask.all():
        return _kernel_numpy(hidden_states, attention_mask,
                             np.asarray(position_ids), wq_kernel, wq_a, wq_b,
                             wk_kernel, wk_a, wk_b, wv_kernel, wv_a, wv_b,
                             wo_kernel, wo_a, wo_b)

    from concourse.bass_utils import run_bass_kernel_spmd

    if _PROG is None:
        _PROG = _build_program()
    nc = _PROG

    in_maps = _host_inputs(hidden_states, position_ids, wq_kernel, wq_a, wq_b,
                           wk_kernel, wk_a, wk_b, wv_kernel, wv_a, wv_b,
                           wo_kernel, wo_a, wo_b)
    trace = bool(int(os.environ.get("BASS_KERNEL_TRACE", "0")))
    res = run_bass_kernel_spmd(nc, in_maps, list(range(N_CORES)), trace=trace)
    _last_exec_time_ns = res.exec_time_ns
    global _last_results
    _last_results = res

    out = np.empty((B, S, HID), np.float32)
    for c in range(N_CORES):
        b, hg = divmod(c, HG)
        yT = res.results[c]["yT"].reshape(NT, QF, TC)  # [chunk, feat, tok]
        y = np.concatenate(list(yT), axis=1)  # [feat, S] bf16
        out[b, :, hg * QF:(hg + 1) * QF] = y.T.astype(np.float32)
    return out


# ---------------- numpy fallback (general shapes / masks) ----------------

def _bf16_round(x):
    return np.asarray(x, np.float32).astype(BF16).astype(np.float32)


def _mm_bf16(x, w):
    y = (np.asarray(x, np.float32).astype(BF16).astype(np.float32)
         @ np.asarray(w, np.float32).astype(BF16).astype(np.float32))
    return _bf16_round(y)


def _lora_dense_np(x, kernel_, a, b):
    y = _mm_bf16(x, kernel_)
    t = _mm_bf16(_mm_bf16(x, a), b)
    return _bf16_round(y + _bf16_round((ALPHA / RANK) * t)).astype(np.float32)


def _kernel_numpy(hidden_states, attention_mask, position_ids, wq_kernel, wq_a,
                  wq_b, wk_kernel, wk_a, wk_b, wv_kernel, wv_a, wv_b, wo_kernel,
                  wo_a, wo_b):
    b, s, _ = hidden_states.shape
    x2 = np.asarray(hidden_states, np.float32).reshape(b * s, HID)
    xq = _lora_dense_np(x2, wq_kernel, wq_a, wq_b).reshape(b, s, H, D)
    xk = _lora_dense_np(x2, wk_kernel, wk_a, wk_b).reshape(b, s, KVH, D)
    xv = _lora_dense_np(x2, wv_kernel, wv_a, wv_b).reshape(b, s, KVH, D)
    g = H // KVH
    xk = np.repeat(xk, g, axis=2)
    xv = np.repeat(xv, g, axis=2)

    inv_freq = 1.0 / THETA ** (np.arange(0, D, 2, dtype=np.float32)[: D // 2]
                               / np.float32(D))
    angles = np.arange(MAX_POS, dtype=np.float32)[:, None] * inv_freq[None, :]
    cos = np.cos(angles).astype(np.float32)[position_ids][:, :, None, :]
    sin = np.sin(angles).astype(np.float32)[position_ids][:, :, None, :]

    def rot(x):
        x = _bf16_round(x)
        xe = x[..., 0::2]
        xo = x[..., 1::2]
        re = xe * cos - xo * sin
        im = xe * sin + xo * cos
        return np.stack((re, im), axis=-1).reshape(x.shape)

    xq, xk = rot(xq), rot(xk)
    causal = np.tril(np.ones((s, s), dtype=bool))
    mask = np.asarray(attention_mask)[:, None, None, :] & causal[None, None]
    bias = np.where(mask, np.float32(0.0), np.float32(np.finfo(np.float32).min))
    scale = np.float32(1.0 / np.sqrt(D))
    qs = _bf16_round(xq * scale)
    ks = _bf16_round(xk)
    out = np.empty((b, s, H, D), np.float32)
    for bi in range(b):
        for h in range(H):
            sc = _bf16_round(qs[bi, :, h, :] @ ks[bi, :, h, :].T) + bias[bi, 0]
            sc = sc - sc.max(axis=-1, keepdims=True)
            e = np.exp(sc)
            attn = e / e.sum(axis=-1, keepdims=True)
            out[bi, :, h, :] = attn @ xv[bi, :, h, :]
    out = out.reshape(b * s, H * D)
    return _lora_dense_np(out, wo_kernel, wo_a, wo_b).reshape(b, s, HID)
